# revision 9
# baseline (speedup 1.0000x reference)
"""CapsLayer2D dynamic-routing kernel for 8 Trainium2 NeuronCores.

Full inputs:  inputs [32,14,14,32,8] f32, W [16,32,8,16] f32
Full output:  out [32,14,14,16,16] f32

Sharding: pure data parallel over batch (4 batches / core -> 784 routing
locations per core). W replicated (host-side: dense [256,256] "sum"
matrix and block-diagonal [256,8192] matrix, both fp16).

v2: fp16 datapath for all the big [112,8192] elementwise work so the
DVE runs in its 2x packed mode; x is DMA'd pre-transposed (no PE
transposes); exp(b) is written j-expanded by the Act engine so the
weighted-sum multiply is fully contiguous; the n-reduction is a tree of
contiguous TT adds instead of a scatter-write + strided reduce.
"""

import sys

sys.path.insert(0, "/opt/trn_rl_repo")

import numpy as np

import concourse.bass as bass
import concourse.mybir as mybir
from concourse.bacc import Bacc
from concourse.tile import TileContext

F32 = mybir.dt.float32
F16 = mybir.dt.float16
ADD = mybir.AluOpType.add
MULT = mybir.AluOpType.mult
SUB = mybir.AluOpType.subtract
MAX = mybir.AluOpType.max
AX = mybir.AxisListType.X
EXP = mybir.ActivationFunctionType.Exp
SQRT = mybir.ActivationFunctionType.Sqrt
SQUARE = mybir.ActivationFunctionType.Square

EPS = 1e-7
B, R, C, N, I = 32, 14, 14, 32, 8
K, J = 16, 16
NCORES = 8
BC = B // NCORES            # batches per core
L = BC * R * C              # 784 locations per core
PT = 112                    # locations per partition-tile
NT = L // PT                # 7 tiles
NI = N * I                  # 256
KJ = K * J                  # 256
KN = K * N                  # 512
NJ = N * J                  # 512
KNJ = K * N * J             # 8192


def _ap(base, dims, off=0):
    """AP over tile `base` ([part, free] contiguous) with free dims
    [(step,count)...] in elements; step 0 = broadcast."""
    return bass.AP(base.tensor, base.offset + off,
                   [list(base.ap[0])] + [list(d) for d in dims])


def build_bass():
    nc = Bacc()
    eps_t = nc.alloc_sbuf_tensor("const-f32-eps", [128, 1], F32)
    nc.gpsimd.memset(eps_t.ap(), EPS)
    nc.const_aps.aps[(F32, EPS)] = eps_t.ap()
    nc.all_engine_barrier()
    xt_d = nc.declare_dram_parameter("xt", [2, 128, L], F16, isOutput=False)
    wsum_d = nc.declare_dram_parameter("wsum", [2, 128, KJ], F16, isOutput=False)
    wbd_d = nc.declare_dram_parameter("wbd", [2, 128, KNJ // 2], F16, isOutput=False)
    out_d = nc.declare_dram_parameter("out", [L, KJ], F32, isOutput=True)

    with TileContext(nc) as tc, nc.allow_low_precision(
            reason="fp16 routing: products/short sums well within 2e-2 budget"):
        import contextlib
        ctx = contextlib.ExitStack()
        with ctx:
            cpool = ctx.enter_context(tc.tile_pool(name="const", bufs=1))
            wpool = ctx.enter_context(tc.tile_pool(name="work", bufs=2))
            bigpool = ctx.enter_context(tc.tile_pool(name="big", bufs=2))
            tmppool = ctx.enter_context(tc.tile_pool(name="tmp", bufs=1))
            pspool = ctx.enter_context(tc.tile_pool(name="ps", bufs=2, space="PSUM"))
            psmm = ctx.enter_context(tc.tile_pool(name="psmm", bufs=4, space="PSUM"))

            wsum0 = cpool.tile([128, KJ], F16)
            wsum1 = cpool.tile([128, KJ], F16)
            wbd0 = cpool.tile([128, KNJ // 2], F16)
            wbd1 = cpool.tile([128, KNJ // 2], F16)
            nc.gpsimd.dma_start(wsum0[:], wsum_d[0])
            nc.gpsimd.dma_start(wsum1[:], wsum_d[1])
            nc.gpsimd.dma_start(wbd0[:], wbd_d[0])
            nc.gpsimd.dma_start(wbd1[:], wbd_d[1])

            # PE warm-up: absorb each const DMA's sem tick into PE's vector
            # clock one at a time, so no later LDWEIGHTS needs >1 sync wait
            # (HW limit: one wait slot on LDW).
            ps_w = pspool.tile([128, 512], F32, tag="psw", name="ps_w")
            for wt in (wsum0, wsum1):
                nc.tensor.matmul(ps_w[:, :KJ], wt[:, :128], wt[:],
                                 start=True, stop=True)
            for wt in (wbd0, wbd1):
                nc.tensor.matmul(ps_w[:], wt[:, :128], wt[:, :512],
                                 start=True, stop=True)

            def squash(s_sb, out_sb, tag):
                """out = squash(s) over j; s_sb [PT,KJ] f32, out_sb [PT,KJ]
                (k-major, dtype of out_sb tile)."""
                sqf = wpool.tile([PT, KJ], F32, tag=f"sqf{tag}", name=f"sqf{tag}")
                sq = wpool.tile([PT, K], F32, tag=f"sq{tag}", name=f"sq{tag}")
                den = wpool.tile([PT, K], F32, tag=f"den{tag}", name=f"den{tag}")
                rt = wpool.tile([PT, K], F32, tag=f"rt{tag}", name=f"rt{tag}")
                q = wpool.tile([PT, K], F32, tag=f"q{tag}", name=f"q{tag}")
                rq = wpool.tile([PT, K], F32, tag=f"rq{tag}", name=f"rq{tag}")
                f = wpool.tile([PT, K], F32, tag=f"f{tag}", name=f"f{tag}")
                nc.scalar.activation(sqf[:], s_sb[:], SQUARE)
                nc.vector.tensor_reduce(
                    sq[:], _ap(sqf, [[J, K], [1, J]]), AX, ADD)
                nc.scalar.add(den[:], sq[:], 1.0)
                nc.scalar.activation(rt[:], sq[:], SQRT, bias=EPS)
                nc.vector.tensor_tensor(q[:], den[:], rt[:], MULT)
                nc.vector.reciprocal(rq[:], q[:])
                nc.vector.tensor_tensor(f[:], sq[:], rq[:], MULT)
                nc.vector.tensor_tensor(
                    _ap(out_sb, [[J, K], [1, J]]),
                    _ap(s_sb, [[J, K], [1, J]]),
                    _ap(f, [[1, K], [0, J]]),
                    MULT)

            for t in range(NT):
                # x transposed halves, DMA'd directly: xt[h] [128, PT] f16
                xt = []
                for h in range(2):
                    xth = wpool.tile([128, PT], F16, tag=f"xT{h}", name=f"xT{h}")
                    nc.gpsimd.dma_start(
                        xth[:], xt_d[h][:, t * PT:(t + 1) * PT])
                    xt.append(xth)

                # predicted p2 [PT, (k n j)] f16 via block-diag W; ch = n-pair
                p2 = bigpool.tile([PT, KNJ], F16, tag="p2", name="p2")
                for ch in range(16):
                    h = ch // 8
                    wb = (wbd0, wbd1)[h]
                    ps = psmm.tile([PT, 512], F32, tag="mm", name="ps_mm")
                    nc.tensor.matmul(
                        ps[:], xt[h][:], wb[:, (ch % 8) * 512:(ch % 8 + 1) * 512],
                        start=True, stop=True)
                    # psum cols (d,k,j) -> p2 cols k*NJ + (2ch+d)*J + j
                    dst = bass.AP(p2.tensor, p2.offset + 2 * ch * J,
                                  [list(p2.ap[0]), [J, 2], [NJ, K], [1, J]])
                    src = _ap(ps, [[KJ, 2], [J, K], [1, J]])
                    nc.scalar.copy(dst, src)

                # iteration 1: c uniform -> s = (x @ wsum)/32
                ps_s = pspool.tile([PT, KJ], F32, tag="s", name="ps_s")
                nc.tensor.matmul(ps_s[:], xt[0][:], wsum0[:], start=True, stop=False)
                nc.tensor.matmul(ps_s[:], xt[1][:], wsum1[:], start=False, stop=True)
                s_sb = wpool.tile([PT, KJ], F32, tag="s_sb", name="s_sb")
                nc.scalar.mul(s_sb[:], ps_s[:], 1.0 / N)
                out_sb = wpool.tile([PT, KJ], F16, tag="out0", name="out_sb")
                squash(s_sb, out_sb, "a")

                b_sb = wpool.tile([PT, KN], F16, tag="b", name="b_sb")
                for it in range(2):
                    # agreement: bn[l,k,n] = sum_j p2[l,k,n,j] * out[l,k,j]
                    tmp = tmppool.tile([PT, KNJ], F16, tag="tmp", name="tmp")
                    nc.vector.tensor_tensor(
                        tmp[:],
                        p2[:],
                        _ap(out_sb, [[J, K], [0, N], [1, J]]),
                        MULT)
                    if it == 0:
                        nc.vector.tensor_reduce(
                            b_sb[:], _ap(tmp, [[J, KN], [1, J]]), AX, ADD)
                    else:
                        bn = wpool.tile([PT, KN], F16, tag="bn", name="bn")
                        nc.vector.tensor_reduce(
                            bn[:], _ap(tmp, [[J, KN], [1, J]]), AX, ADD)
                        nc.vector.tensor_tensor(b_sb[:], b_sb[:], bn[:], ADD)
                    # softmax over n, max-subtracted per (l,k) so exp fits in
                    # fp16 (normalization cancels the shift exactly).
                    # e_rep = exp(b-bmax) expanded along j on the Act engine so
                    # the weighted-sum multiply below is contiguous fp16 (2x).
                    bmax = wpool.tile([PT, K], F16, tag="bmax", name="bmax")
                    nc.vector.tensor_reduce(
                        bmax[:], _ap(b_sb, [[N, K], [1, N]]), AX, MAX)
                    bs = wpool.tile([PT, KN], F16, tag="bs", name="bs")
                    nc.vector.tensor_tensor(
                        bs[:], b_sb[:],
                        _ap(bmax, [[1, K], [0, N]]), SUB)
                    e_rep = bigpool.tile([PT, KNJ], F16, tag="erep", name="e_rep")
                    nc.scalar.activation(
                        _ap(e_rep, [[J, KN], [1, J]]),
                        _ap(bs, [[1, KN], [0, J]]),
                        EXP)
                    e_sb = wpool.tile([PT, KN], F16, tag="e", name="e_sb")
                    nc.scalar.activation(e_sb[:], bs[:], EXP)
                    se = wpool.tile([PT, K], F32, tag="se", name="se")
                    nc.vector.tensor_reduce(
                        se[:], _ap(e_sb, [[N, K], [1, N]]), AX, ADD)
                    r = wpool.tile([PT, K], F32, tag="r", name="r")
                    nc.vector.reciprocal(r[:], se[:])
                    # ws[l,k,j] = sum_n e[l,k,n]*p2[l,k,n,j]:
                    # contiguous multiply, then tree-halve over n.
                    tmp2 = tmppool.tile([PT, KNJ], F16, tag="tmp2", name="tmp2")
                    nc.vector.tensor_tensor(tmp2[:], p2[:], e_rep[:], MULT)
                    tree = tmppool.tile([PT, 7680], F16, tag="tree", name="tree")
                    ws = wpool.tile([PT, KJ], F32, tag="ws", name="ws")
                    src_t, src_off = tmp2, 0
                    dst_off = 0
                    for lvl in range(5):
                        G = NJ >> lvl                   # 512,256,128,64,32
                        if lvl < 4:
                            dst = _ap(tree, [[G // 2, K], [1, G // 2]], dst_off)
                        else:
                            dst = _ap(ws, [[J, K], [1, J]])
                        nc.vector.tensor_tensor(
                            dst,
                            _ap(src_t, [[G, K], [1, G // 2]], src_off),
                            _ap(src_t, [[G, K], [1, G // 2]], src_off + G // 2),
                            ADD)
                        src_t, src_off = tree, dst_off
                        dst_off += (G // 2) * K
                    # s = ws * (1/sum e), then squash
                    s2 = wpool.tile([PT, KJ], F32, tag="s2", name="s2")
                    nc.vector.tensor_tensor(
                        _ap(s2, [[J, K], [1, J]]),
                        _ap(ws, [[J, K], [1, J]]),
                        _ap(r, [[1, K], [0, J]]),
                        MULT)
                    if it == 0:
                        out_sb = wpool.tile([PT, KJ], F16, tag="out1",
                                            name="out_it")
                    else:
                        out_sb = wpool.tile([PT, KJ], F32, tag="out2",
                                            name="out_fin")
                    squash(s2, out_sb, f"i{it}")

                nc.gpsimd.dma_start(out_d[t * PT:(t + 1) * PT, :], out_sb[:])
    nc.compile()
    return nc


def host_prep(inputs, W):
    x = np.ascontiguousarray(inputs, np.float32).reshape(NCORES, L, NI)
    xt = np.ascontiguousarray(
        x.transpose(0, 2, 1), np.float16).reshape(NCORES, 2, 128, L)
    wsum = np.ascontiguousarray(
        W.transpose(1, 2, 0, 3).reshape(NI, KJ), np.float16).reshape(2, 128, KJ)
    wbd_full = np.zeros((NI, KNJ), np.float16)
    for n in range(N):
        wbd_full[n * I:(n + 1) * I, n * KJ:(n + 1) * KJ] = (
            W[:, n].transpose(1, 0, 2).reshape(I, KJ))
    wbd = np.stack([wbd_full[0:128, 0:KNJ // 2],
                    wbd_full[128:256, KNJ // 2:]])
    return xt, wsum, wbd


_CACHED = {}


def kernel(inputs, W):
    from concourse.bass_utils import run_bass_kernel_spmd

    xt, wsum, wbd = host_prep(inputs, W)
    if "nc" not in _CACHED:
        _CACHED["nc"] = build_bass()
    nc = _CACHED["nc"]
    in_maps = [{"xt": np.ascontiguousarray(xt[c]), "wsum": wsum, "wbd": wbd}
               for c in range(NCORES)]
    res = run_bass_kernel_spmd(nc, in_maps, core_ids=list(range(NCORES)))
    out = np.stack([res.results[c]["out"] for c in range(NCORES)])
    return out.reshape(B, R, C, K, J)


# revision 10
# speedup vs baseline: 1.2818x; 1.2818x over previous
"""CapsLayer2D dynamic-routing kernel for 8 Trainium2 NeuronCores.

Full inputs:  inputs [32,14,14,32,8] f32, W [16,32,8,16] f32
Full output:  out [32,14,14,16,16] f32

Sharding: pure data parallel over batch (4 batches / core -> 784 routing
locations per core). W replicated (host-side: dense [256,256] "sum"
matrix and block-diagonal [256,8192] matrix, both fp16).

v2: fp16 datapath for all the big [112,8192] elementwise work so the
DVE runs in its 2x packed mode; x is DMA'd pre-transposed (no PE
transposes); exp(b) is written j-expanded by the Act engine so the
weighted-sum multiply is fully contiguous; the n-reduction is a tree of
contiguous TT adds instead of a scatter-write + strided reduce.
"""

import sys

sys.path.insert(0, "/opt/trn_rl_repo")

import numpy as np

import concourse.bass as bass
import concourse.mybir as mybir
from concourse.bacc import Bacc
from concourse.tile import TileContext

F32 = mybir.dt.float32
F16 = mybir.dt.float16
ADD = mybir.AluOpType.add
MULT = mybir.AluOpType.mult
SUB = mybir.AluOpType.subtract
MAX = mybir.AluOpType.max
AX = mybir.AxisListType.X
EXP = mybir.ActivationFunctionType.Exp
SQRT = mybir.ActivationFunctionType.Sqrt
SQUARE = mybir.ActivationFunctionType.Square

EPS = 1e-7
B, R, C, N, I = 32, 14, 14, 32, 8
K, J = 16, 16
NCORES = 8
BC = B // NCORES            # batches per core
L = BC * R * C              # 784 locations per core
PT = 112                    # locations per partition-tile
NT = L // PT                # 7 tiles
NI = N * I                  # 256
KJ = K * J                  # 256
KN = K * N                  # 512
NJ = N * J                  # 512
KNJ = K * N * J             # 8192


def _ap(base, dims, off=0):
    """AP over tile `base` ([part, free] contiguous) with free dims
    [(step,count)...] in elements; step 0 = broadcast."""
    return bass.AP(base.tensor, base.offset + off,
                   [list(base.ap[0])] + [list(d) for d in dims])


def build_bass():
    nc = Bacc()
    eps_t = nc.alloc_sbuf_tensor("const-f32-eps", [128, 1], F32)
    nc.gpsimd.memset(eps_t.ap(), EPS)
    nc.const_aps.aps[(F32, EPS)] = eps_t.ap()
    nc.all_engine_barrier()
    xt_d = nc.declare_dram_parameter("xt", [2, 128, L], F16, isOutput=False)
    wsum_d = nc.declare_dram_parameter("wsum", [2, 128, KJ], F16, isOutput=False)
    wbd_d = nc.declare_dram_parameter("wbd", [2, 128, KNJ // 2], F16, isOutput=False)
    out_d = nc.declare_dram_parameter("out", [L, KJ], F32, isOutput=True)

    with TileContext(nc) as tc, nc.allow_low_precision(
            reason="fp16 routing: products/short sums well within 2e-2 budget"):
        import contextlib
        ctx = contextlib.ExitStack()
        with ctx:
            cpool = ctx.enter_context(tc.tile_pool(name="const", bufs=1))
            wpool = ctx.enter_context(tc.tile_pool(name="work", bufs=2))
            bigpool = ctx.enter_context(tc.tile_pool(name="big", bufs=2))
            tmppool = ctx.enter_context(tc.tile_pool(name="tmp", bufs=1))
            pspool = ctx.enter_context(tc.tile_pool(name="ps", bufs=2, space="PSUM"))
            psmm = ctx.enter_context(tc.tile_pool(name="psmm", bufs=4, space="PSUM"))

            wsum0 = cpool.tile([128, KJ], F16)
            wsum1 = cpool.tile([128, KJ], F16)
            wbd0 = cpool.tile([128, KNJ // 2], F16)
            wbd1 = cpool.tile([128, KNJ // 2], F16)
            nc.gpsimd.dma_start(wsum0[:], wsum_d[0])
            nc.gpsimd.dma_start(wsum1[:], wsum_d[1])
            nc.gpsimd.dma_start(wbd0[:], wbd_d[0])
            nc.gpsimd.dma_start(wbd1[:], wbd_d[1])

            # PE warm-up: absorb each const DMA's sem tick into PE's vector
            # clock one at a time, so no later LDWEIGHTS needs >1 sync wait
            # (HW limit: one wait slot on LDW).
            ps_w = pspool.tile([128, 512], F32, tag="psw", name="ps_w")
            for wt in (wsum0, wsum1):
                nc.tensor.matmul(ps_w[:, :KJ], wt[:, :128], wt[:],
                                 start=True, stop=True)
            for wt in (wbd0, wbd1):
                nc.tensor.matmul(ps_w[:], wt[:, :128], wt[:, :512],
                                 start=True, stop=True)

            def squash(s_sb, out_sb, tag):
                """out = squash(s) over j; s_sb [PT,KJ] f32, out_sb [PT,KJ]
                (k-major, dtype of out_sb tile)."""
                sqf = wpool.tile([PT, KJ], F32, tag=f"sqf{tag}", name=f"sqf{tag}")
                sq = wpool.tile([PT, K], F32, tag=f"sq{tag}", name=f"sq{tag}")
                den = wpool.tile([PT, K], F32, tag=f"den{tag}", name=f"den{tag}")
                rt = wpool.tile([PT, K], F32, tag=f"rt{tag}", name=f"rt{tag}")
                q = wpool.tile([PT, K], F32, tag=f"q{tag}", name=f"q{tag}")
                rq = wpool.tile([PT, K], F32, tag=f"rq{tag}", name=f"rq{tag}")
                f = wpool.tile([PT, K], F32, tag=f"f{tag}", name=f"f{tag}")
                nc.scalar.activation(sqf[:], s_sb[:], SQUARE)
                nc.vector.tensor_reduce(
                    sq[:], _ap(sqf, [[J, K], [1, J]]), AX, ADD)
                nc.scalar.add(den[:], sq[:], 1.0)
                nc.scalar.activation(rt[:], sq[:], SQRT, bias=EPS)
                nc.vector.tensor_tensor(q[:], den[:], rt[:], MULT)
                nc.vector.reciprocal(rq[:], q[:])
                nc.vector.tensor_tensor(f[:], sq[:], rq[:], MULT)
                nc.vector.tensor_tensor(
                    _ap(out_sb, [[J, K], [1, J]]),
                    _ap(s_sb, [[J, K], [1, J]]),
                    _ap(f, [[1, K], [0, J]]),
                    MULT)

            for t in range(NT):
                # x transposed halves, DMA'd directly: xt[h] [128, PT] f16
                xt = []
                for h in range(2):
                    xth = wpool.tile([128, PT], F16, tag=f"xT{h}", name=f"xT{h}")
                    nc.gpsimd.dma_start(
                        xth[:], xt_d[h][:, t * PT:(t + 1) * PT])
                    xt.append(xth)

                # predicted p2 [PT, (k n j)] f16 via block-diag W; ch = n-pair
                p2 = bigpool.tile([PT, KNJ], F16, tag="p2", name="p2")
                for ch in range(16):
                    h = ch // 8
                    wb = (wbd0, wbd1)[h]
                    ps = psmm.tile([PT, 512], F32, tag="mm", name="ps_mm")
                    nc.tensor.matmul(
                        ps[:], xt[h][:], wb[:, (ch % 8) * 512:(ch % 8 + 1) * 512],
                        start=True, stop=True)
                    # psum cols (d,k,j) -> p2 cols k*NJ + (2ch+d)*J + j
                    dst = bass.AP(p2.tensor, p2.offset + 2 * ch * J,
                                  [list(p2.ap[0]), [J, 2], [NJ, K], [1, J]])
                    src = _ap(ps, [[KJ, 2], [J, K], [1, J]])
                    nc.scalar.copy(dst, src)

                # iteration 1: c uniform -> s = (x @ wsum)/32
                ps_s = pspool.tile([PT, KJ], F32, tag="s", name="ps_s")
                nc.tensor.matmul(ps_s[:], xt[0][:], wsum0[:], start=True, stop=False)
                nc.tensor.matmul(ps_s[:], xt[1][:], wsum1[:], start=False, stop=True)
                s_sb = wpool.tile([PT, KJ], F32, tag="s_sb", name="s_sb")
                nc.scalar.mul(s_sb[:], ps_s[:], 1.0 / N)
                out_sb = wpool.tile([PT, KJ], F16, tag="out0", name="out_sb")
                squash(s_sb, out_sb, "a")

                b_sb = wpool.tile([PT, KN], F16, tag="b", name="b_sb")
                for it in range(2):
                    # agreement: bn[l,k,n] = sum_j p2[l,k,n,j] * out[l,k,j]
                    tmp = tmppool.tile([PT, KNJ], F16, tag="tmp", name="tmp")
                    nc.vector.tensor_tensor(
                        tmp[:],
                        p2[:],
                        _ap(out_sb, [[J, K], [0, N], [1, J]]),
                        MULT)
                    # tree-halve over j (TR doesn't engage the DVE 2x mode)
                    bn = b_sb if it == 0 else wpool.tile(
                        [PT, KN], F16, tag="bn", name="bn")
                    tree = tmppool.tile([PT, 7680], F16, tag="tree", name="tree")
                    src_t, src_off = tmp, 0
                    dst_off = 0
                    for lvl in range(4):
                        G = J >> lvl                    # 16,8,4,2
                        if lvl < 3:
                            dst = _ap(tree, [[G // 2, KN], [1, G // 2]], dst_off)
                            nc.vector.tensor_tensor(
                                dst,
                                _ap(src_t, [[G, KN], [1, G // 2]], src_off),
                                _ap(src_t, [[G, KN], [1, G // 2]],
                                    src_off + G // 2),
                                ADD)
                        else:
                            nc.vector.tensor_tensor(
                                bn[:],
                                _ap(src_t, [[2, KN]], src_off),
                                _ap(src_t, [[2, KN]], src_off + 1),
                                ADD)
                        src_t, src_off = tree, dst_off
                        dst_off += (G // 2) * KN
                    if it == 1:
                        nc.vector.tensor_tensor(b_sb[:], b_sb[:], bn[:], ADD)
                    # softmax over n, max-subtracted per (l,k) so exp fits in
                    # fp16 (normalization cancels the shift exactly).
                    # e2 = exp(b-bmax) duplicated over j-pairs on Act so the
                    # weighted-sum multiply runs contiguous fp16 (DVE 2x).
                    bmax = wpool.tile([PT, K], F16, tag="bmax", name="bmax")
                    nc.vector.tensor_reduce(
                        bmax[:], _ap(b_sb, [[N, K], [1, N]]), AX, MAX)
                    bs = wpool.tile([PT, KN], F16, tag="bs", name="bs")
                    nc.vector.tensor_tensor(
                        bs[:], b_sb[:],
                        _ap(bmax, [[1, K], [0, N]]), SUB)
                    e2 = wpool.tile([PT, 2 * KN], F16, tag="e2", name="e2")
                    nc.scalar.activation(
                        _ap(e2, [[2, KN], [1, 2]]),
                        _ap(bs, [[1, KN], [0, 2]]),
                        EXP)
                    se = wpool.tile([PT, K], F32, tag="se", name="se")
                    nc.vector.tensor_reduce(
                        se[:], _ap(e2, [[2 * N, K], [2, N]]), AX, ADD)
                    r = wpool.tile([PT, K], F32, tag="r", name="r")
                    nc.vector.reciprocal(r[:], se[:])
                    # ws[l,k,j] = sum_n e[l,k,n]*p2[l,k,n,j]:
                    # contiguous multiply, then tree-halve over n.
                    tmp2 = tmppool.tile([PT, KNJ], F16, tag="tmp2", name="tmp2")
                    nc.vector.tensor_tensor(
                        tmp2[:], p2[:],
                        _ap(e2, [[2, KN], [0, J // 2], [1, 2]]),
                        MULT)
                    tree2 = tmppool.tile([PT, 7680], F16, tag="tree2",
                                         name="tree2")
                    ws = wpool.tile([PT, KJ], F32, tag="ws", name="ws")
                    src_t, src_off = tmp2, 0
                    dst_off = 0
                    for lvl in range(5):
                        G = NJ >> lvl                   # 512,256,128,64,32
                        if lvl < 4:
                            dst = _ap(tree2, [[G // 2, K], [1, G // 2]], dst_off)
                        else:
                            dst = _ap(ws, [[J, K], [1, J]])
                        nc.vector.tensor_tensor(
                            dst,
                            _ap(src_t, [[G, K], [1, G // 2]], src_off),
                            _ap(src_t, [[G, K], [1, G // 2]], src_off + G // 2),
                            ADD)
                        src_t, src_off = tree2, dst_off
                        dst_off += (G // 2) * K
                    # s = ws * (1/sum e), then squash
                    s2 = wpool.tile([PT, KJ], F32, tag="s2", name="s2")
                    nc.vector.tensor_tensor(
                        _ap(s2, [[J, K], [1, J]]),
                        _ap(ws, [[J, K], [1, J]]),
                        _ap(r, [[1, K], [0, J]]),
                        MULT)
                    if it == 0:
                        out_sb = wpool.tile([PT, KJ], F16, tag="out1",
                                            name="out_it")
                    else:
                        out_sb = wpool.tile([PT, KJ], F32, tag="out2",
                                            name="out_fin")
                    squash(s2, out_sb, f"i{it}")

                nc.gpsimd.dma_start(out_d[t * PT:(t + 1) * PT, :], out_sb[:])
    nc.compile()
    return nc


def host_prep(inputs, W):
    x = np.ascontiguousarray(inputs, np.float32).reshape(NCORES, L, NI)
    xt = np.ascontiguousarray(
        x.transpose(0, 2, 1), np.float16).reshape(NCORES, 2, 128, L)
    wsum = np.ascontiguousarray(
        W.transpose(1, 2, 0, 3).reshape(NI, KJ), np.float16).reshape(2, 128, KJ)
    wbd_full = np.zeros((NI, KNJ), np.float16)
    for n in range(N):
        wbd_full[n * I:(n + 1) * I, n * KJ:(n + 1) * KJ] = (
            W[:, n].transpose(1, 0, 2).reshape(I, KJ))
    wbd = np.stack([wbd_full[0:128, 0:KNJ // 2],
                    wbd_full[128:256, KNJ // 2:]])
    return xt, wsum, wbd


_CACHED = {}


def kernel(inputs, W):
    from concourse.bass_utils import run_bass_kernel_spmd

    xt, wsum, wbd = host_prep(inputs, W)
    if "nc" not in _CACHED:
        _CACHED["nc"] = build_bass()
    nc = _CACHED["nc"]
    in_maps = [{"xt": np.ascontiguousarray(xt[c]), "wsum": wsum, "wbd": wbd}
               for c in range(NCORES)]
    res = run_bass_kernel_spmd(nc, in_maps, core_ids=list(range(NCORES)))
    out = np.stack([res.results[c]["out"] for c in range(NCORES)])
    return out.reshape(B, R, C, K, J)
